# revision 2
# baseline (speedup 1.0000x reference)
"""Trainium2 Bass kernel for nn_Logic_Model_80607946211458 (v2).

Math (per batch row b; formulas i=0,1):
  ind_j = (ds_j <= t);  D'_i = <ind, A_i> + (A_i30 + A_i31 - 3)
  feat_i = exp(-|D'_i| / sigma)
  Mb_i = max_j ind_j * ds_j * A_ij ;  sigm_i = 1/(1+exp(Mb_i - t))
  col_i = num_i / den_i  (softmin over valid pairs, piecewise-const in td)
  sg_i = sigm_i * feat_i * col_i * fw_i
  out_i = ln((b0+sg_i)*pi_i) - b0*t + (sigm_i*Mb_i - t)*sg_i
  out_0col = ln(b0) + ln(pi0) - b0*t

Device design (per core, P=63 batch rows on partitions):
* The piecewise-constant softmin terms are folded into per-COLUMN host
  constants: a 28-col block (4 groups of 7: [den_f0|den_f1|num_f0|num_f1],
  each = 3 "+TOL" pair cols + 3 swapped "-TOL" cols + 1 always-on base col)
  so den/num fall out of ONE stt compare, ONE tt mult and ONE grouped
  reduce.  fw and the pair-validity mask are folded into the K columns;
  degenerate formulas (no valid pairs) fold to col=1 via the base cols.
* ec = A30+A31-3 folds into the D reduce as an extra always-on column.
* Both divisions (1/den and sigmoid's 1/(1+e1)) ride ONE single-iteration
  Newton reciprocal over a packed [P,4] tile (exponent-flip magic seed,
  max rel err ~1.2e-3 vs the 2e-2 gate; custom-DVE ops are rejected by
  this walrus).  The Newton sign flip (Y1 = -1/Q) cancels in
  sg = (num*Y1den)*(feat*Y1sig) and is absorbed into the tail algebra.
* ACT does the three transcendentals (exp, exp, ln) from the single
  natural_log_exp table (one table load, hidden under the input DMA).
"""

import sys

import numpy as np

if "/opt/trn_rl_repo" not in sys.path:
    sys.path.insert(0, "/opt/trn_rl_repo")

import concourse.bass as bass
import concourse.mybir as mybir
from concourse.bass_utils import run_bass_kernel_spmd


def _ensure_axon_hooks():
    """Provide ``antenv.axon_hooks`` if the image lacks it."""
    try:
        import antenv.axon_hooks  # noqa: F401
        return
    except ImportError:
        pass
    try:
        import antenv
    except ImportError:
        return
    import types

    mod = types.ModuleType("antenv.axon_hooks")
    holder = {"hook": None, "tried": False}

    def set_axon_ntff_profile_hook(h):
        holder["hook"] = h
        holder["tried"] = True

    def get_axon_ntff_profile_hook():
        if holder["hook"] is None and not holder["tried"]:
            holder["tried"] = True
            try:
                from trn_agent_boot.trn_boot import _ntff_profile_via_ctypes
                holder["hook"] = _ntff_profile_via_ctypes(
                    "/opt/axon/libaxon_pjrt.so")
            except Exception:
                holder["hook"] = None
        return holder["hook"]

    mod.set_axon_ntff_profile_hook = set_axon_ntff_profile_hook
    mod.get_axon_ntff_profile_hook = get_axon_ntff_profile_hook
    sys.modules["antenv.axon_hooks"] = mod
    antenv.axon_hooks = mod


_ensure_axon_hooks()

NCORES = 8
NB = 30
KSEL = 3
SIGMA = 0.1
TEMP = 0.07
TOL = 0.02
_PA = np.array([0, 0, 1])
_PB = np.array([1, 2, 2])

# ---- packed input column layout (all float32) ----
W2 = 2 * (NB + 1)    # 62: [x(30), extra, x(30), extra]
C_DS2 = 0            # 62: ds | sentinel(-1e30) | ds | sentinel
C_A2 = 62            # 62: A0 | ec0 | A1 | ec1  (broadcast row), scaled 1.0
C_DSA = 124          # 62: ds*A0 | 0 | ds*A1 | 0
C_P28 = 186          # 28: compare P side
C_Q28 = 214          # 28: compare Q side
C_K28 = 242          # 28: per-col softmin constants (broadcast row)
C_T = 270            # 1: t
C_NT = 271           # 1: -t  (ACT bias for e1)
C_TB = 272           # 1: -b0*t
C_M1 = 273           # 1: -1.0
C_PI = 274           # 2: pi[1:]
C_MG = 276           # 4: int32 0x7EF127EA magic bits (reciprocal seed)
C_KE = 280           # 28: boundary-correction constants (only if needed)
NCOL_FAST = 280
NCOL_BND = 308

F32 = mybir.dt.float32
I32 = mybir.dt.int32
ALU = mybir.AluOpType
ACTF = mybir.ActivationFunctionType
MAGIC = 0x7EF127EA

_BUILD_CACHE: dict = {}
LAST_RESULT = None


def _rrf_region_value(j: int, prob: np.ndarray) -> float:
    """softmin input value when td falls in region j (0:>TOL, 1:|td|<TOL,
    2:<-TOL, -1: exactly on a boundary), in float64."""
    p = prob.astype(np.float64)
    c = np.zeros(3, np.float64)
    if j >= 0:
        c[j] = 1.0
    c3 = 1.0 - p[0] * c[0] - p[1] * c[1] - p[2] * c[2]
    tbi = np.array([c[0], c[1], c[2], c3], np.float64)
    u = tbi * p
    w = np.exp(u / TEMP)
    return float((w * u).sum() / w.sum())


def _f32(x) -> float:
    return float(np.float32(x))


def _build(cfg):
    (P, need_boundary, neg_inv_sigma, b0, lp0c) = cfg
    ncol = NCOL_BND if need_boundary else NCOL_FAST

    from contextlib import ExitStack

    ctx = ExitStack()
    nc = bass.Bass()
    # Only the SP hardware DGE queue is used; dropping the unused queue
    # declarations shortens the NEFF's end-of-program queue/event teardown.
    nc.m.queues = [q for q in nc.m.queues if q.name == "qSPDynamicHW"]
    xd = nc.dram_tensor("x", [P, ncol], F32, kind="ExternalInput")
    od = nc.dram_tensor("o", [P, 3], F32, kind="ExternalOutput")

    sb = lambda name, shape: ctx.enter_context(nc.sbuf_tensor(name, shape, F32))
    sem = lambda name: ctx.enter_context(nc.semaphore(name))

    X = sb("xt", [P, ncol])
    q01 = sb("q01", [P, W2])
    mm = sb("mm", [P, W2])
    D2 = sb("d2", [P, 2])
    Mb = sb("mbt", [P, 2])
    ab = sb("ab", [P, 2])
    m28 = sb("m28", [P, 28])
    v28 = sb("v28", [P, 28])
    if need_boundary:
        m28e = sb("m28e", [P, 28])
        v28e = sb("v28e", [P, 28])
        v28t = sb("v28t", [P, 28])
    # Q4 = [den0, den1, e1p0, e1p1]; NF4 = [num0, num1, feat0, feat1]
    Q4 = sb("q4", [P, 4])
    NF4 = sb("nf4", [P, 4])
    e1 = sb("e1", [P, 2])
    Y0 = sb("y0", [P, 4])
    T1 = sb("t1", [P, 4])
    Y1 = sb("y1", [P, 4])
    P4 = sb("p4", [P, 4])
    sm = sb("sm", [P, 2])
    sg = sb("sg", [P, 2])
    cur2 = sb("cur2", [P, 2])
    xx = sb("xx", [P, 2])
    xx2 = sb("xx2", [P, 2])
    lcur = sb("lcur", [P, 2])
    O = sb("ot", [P, 3])
    de_o = sb("de_o", [P, 1])
    dum_in = nc.const_aps.aps[(F32, 1.0)].tensor[0:P, 0:1]

    dma_in = sem("dma_in")
    dma_out = sem("dma_out")
    v1 = sem("v1")
    v6 = sem("v6")
    a1 = sem("a1")
    v2 = sem("v2")
    a2 = sem("a2")
    cdone = sem("cdone")

    tS = X[:, C_T:C_T + 1]

    # Issue the input DMA straight from the main BB; it is hoisted ahead of
    # the framework preamble barrier below so descriptor generation and the
    # transfer overlap the barrier instead of following it.
    dma_in_inst = nc.sync.dma_start(out=X[:], in_=xd[:]).then_inc(dma_in, 16)

    with nc.Block(no_gpsimd_drain=True) as block:

        @block.sync
        def _(sync):
            sync.wait_ge(cdone, 1)
            sync.dma_start(out=od[:], in_=O[:]).then_inc(dma_out, 16)

        @block.vector
        def _(vector):
            v = nc.vector
            v.wait_ge(dma_in, 16)
            # g1 -- feed the Mb -> ACT e1 path first
            v.scalar_tensor_tensor(
                out=q01[:], in0=X[:, C_DS2:C_DS2 + W2], scalar=tS,
                in1=X[:, C_A2:C_A2 + W2], op0=ALU.is_le, op1=ALU.mult)
            v.scalar_tensor_tensor(
                out=mm[:], in0=X[:, C_DS2:C_DS2 + W2], scalar=tS,
                in1=X[:, C_DSA:C_DSA + W2], op0=ALU.is_le, op1=ALU.mult)
            v.drain(fusable=True)
            # g2a: Mb unlocks ACT's e1 as early as possible (sem rides the
            # reduce itself -- it fires at completion, when Mb is written)
            v.tensor_reduce(
                out=Mb[:], in_=mm[:].rearrange("p (f j) -> p f j", j=NB + 1),
                axis=mybir.AxisListType.X, op=ALU.max).then_inc(v1, 1)
            # g2b (in ACT shadow)
            v.tensor_reduce(
                out=D2[:], in_=q01[:].rearrange("p (f j) -> p f j", j=NB + 1),
                axis=mybir.AxisListType.X, op=ALU.add)
            v.scalar_tensor_tensor(
                out=m28[:], in0=X[:, C_P28:C_P28 + 28], scalar=_f32(-TOL),
                in1=X[:, C_Q28:C_Q28 + 28], op0=ALU.add, op1=ALU.is_gt)
            if need_boundary:
                v.scalar_tensor_tensor(
                    out=m28e[:], in0=X[:, C_P28:C_P28 + 28], scalar=_f32(-TOL),
                    in1=X[:, C_Q28:C_Q28 + 28], op0=ALU.add,
                    op1=ALU.is_equal)
            v.drain(fusable=True)
            # g3 -- feat unblocks on the ab op itself
            v.scalar_tensor_tensor(
                out=ab[:], in0=D2[:], scalar=X[:, C_M1:C_M1 + 1],
                in1=D2[:], op0=ALU.mult, op1=ALU.max).then_inc(v6, 1)
            v.tensor_mul(out=v28[:], in0=m28[:], in1=X[:, C_K28:C_K28 + 28])
            if need_boundary:
                v.tensor_mul(out=v28e[:], in0=m28e[:],
                             in1=X[:, C_KE:C_KE + 28])
            v.drain(fusable=True)
            # g4: e1p right after ACT's e1 lands, in the feat shadow
            v.wait_ge(a1, 1)
            v.tensor_scalar_add(out=Q4[:, 2:4], in0=e1[:], scalar1=1.0)
            if need_boundary:
                v.tensor_add(out=v28t[:], in0=v28[:], in1=v28e[:])
                vsrc = v28t
            else:
                vsrc = v28
            v.drain(fusable=True)
            # g3b (in ACT shadow)
            v.tensor_reduce(
                out=Q4[:, 0:2],
                in_=vsrc[:, 0:14].rearrange("p (g k) -> p g k", k=7),
                axis=mybir.AxisListType.X, op=ALU.add)
            v.tensor_reduce(
                out=NF4[:, 0:2],
                in_=vsrc[:, 14:28].rearrange("p (g k) -> p g k", k=7),
                axis=mybir.AxisListType.X, op=ALU.add)
            v.drain(fusable=True)
            # g5..g7: one-iteration Newton 1/Q (sign-flipped: Y1 = -1/Q)
            v.tensor_sub(out=Y0[:].bitcast(I32),
                         in0=X[:, C_MG:C_MG + 4].bitcast(I32),
                         in1=Q4[:].bitcast(I32))
            v.drain(fusable=True)
            v.tensor_mul(out=T1[:], in0=Q4[:], in1=Y0[:])
            v.drain(fusable=True)
            v.scalar_tensor_tensor(out=Y1[:], in0=T1[:], scalar=-2.0,
                                   in1=Y0[:], op0=ALU.add, op1=ALU.mult)
            v.drain(fusable=True)
            # g8: P4 = [-num*rden | -feat*sigm];  sm = -sigm*Mb
            v.wait_ge(a1, 2)  # feat written into NF4[:, 2:4]
            v.tensor_mul(out=P4[:], in0=Y1[:], in1=NF4[:])
            v.tensor_mul(out=sm[:], in0=Y1[:, 2:4], in1=Mb[:])
            v.drain(fusable=True)
            # g9: the two Newton sign flips cancel
            v.tensor_mul(out=sg[:], in0=P4[:, 0:2], in1=P4[:, 2:4])
            v.drain(fusable=True)
            # g10: cur2 first -- the ACT Ln chain is the critical consumer
            v.scalar_tensor_tensor(out=cur2[:], in0=sg[:], scalar=b0,
                                   in1=X[:, C_PI:C_PI + 2],
                                   op0=ALU.add, op1=ALU.mult).then_inc(v2, 1)
            v.scalar_tensor_tensor(out=xx[:], in0=sm[:], scalar=tS,
                                   in1=sg[:], op0=ALU.add, op1=ALU.mult)
            v.drain(fusable=True)
            # g11 (overlaps ACT Ln): xx2 = xx - (-b0*t); also log_p0 column
            v.tensor_scalar_sub(out=xx2[:], in0=xx[:],
                                scalar1=X[:, C_TB:C_TB + 1])
            v.tensor_scalar(out=O[:, 0:1], in0=tS, scalar1=-b0,
                            scalar2=lp0c, op0=ALU.mult, op1=ALU.add)
            v.drain(fusable=True)
            # g12: out = lcur - xx2  (cdone rides the op; the Pool DMA won't
            # read O for another ~1.4us of descriptor-gen + DGE delay)
            v.wait_ge(a2, 1)
            v.tensor_sub(out=O[:, 1:3], in0=lcur[:],
                         in1=xx2[:]).then_inc(cdone, 1)

        @block.scalar
        def _(scalar):
            s = nc.scalar
            # preload the exp/ln table while the input DMA flies
            s.activation(de_o[:], dum_in, ACTF.Exp)
            s.wait_ge(v1, 1)
            s.activation(e1[:], Mb[:], ACTF.Exp,
                         bias=X[:, C_NT:C_NT + 1]).then_inc(a1, 1)
            s.wait_ge(v6, 1)
            s.activation(NF4[:, 2:4], ab[:], ACTF.Exp,
                         scale=neg_inv_sigma).then_inc(a1, 1)
            s.wait_ge(v2, 1)
            s.activation(lcur[:], cur2[:], ACTF.Ln).then_inc(a2, 1)

    # Hoist the input DMA ahead of the preamble all-engine barrier in the
    # main BB: it has no dependency on the const-AP memsets, and SP reaching
    # it ~1us earlier pulls the whole DMA->compute chain forward.
    bb = nc.main_func.blocks[0]
    insts = list(bb.instructions)
    di = next(i for i, x in enumerate(insts)
              if x.name == dma_in_inst.ins.name)
    dma = insts.pop(di)
    bi = next(i for i, x in enumerate(insts)
              if type(x).__name__ == "InstDrain")
    insts.insert(bi, dma)
    del bb.instructions[:]
    bb.instructions.extend(insts)

    nc.finalize()
    return nc, ctx


def _prepare(t, data_sample, pi, A, base, formula_weight, prob):
    t = np.asarray(t, np.float32)
    ds = np.asarray(data_sample, np.float32)
    pi = np.asarray(pi, np.float32)
    A = np.asarray(A, np.float32)
    base = np.asarray(base, np.float32)
    fw = np.asarray(formula_weight, np.float32)
    prob = np.asarray(prob, np.float32)

    B = t.shape[0]
    P = -(-B // NCORES)
    nF = A.shape[0]
    assert nF == 2 and ds.shape[1] == NB and A.shape[1] == NB + 2

    # --- A top-k bookkeeping ---
    p_all = np.zeros(6, np.int64)
    q_all = np.zeros(6, np.int64)
    pv = np.zeros(6, np.float32)
    sel = np.zeros(2, np.float32)
    for i in range(nF):
        idx = np.argsort(-A[i], kind="stable")[:KSEL]
        idx = np.sort(idx)
        valid = idx < NB
        pvi = (valid[_PA] & valid[_PB]).astype(np.float32)
        pv[3 * i:3 * i + 3] = pvi
        p_all[3 * i:3 * i + 3] = np.minimum(idx[_PA], NB - 1)
        q_all[3 * i:3 * i + 3] = np.minimum(idx[_PB], NB - 1)
        sel[i] = 1.0 if pvi.sum() > 0 else 0.0
    nv = pv.reshape(2, 3).sum(1)

    # --- piecewise-constant softmin values ---
    R = [_rrf_region_value(j, prob) for j in (0, 1, 2, -1)]
    aR = np.array([np.exp(-r / TEMP) for r in R], np.float64)
    bR = aR * np.array(R, np.float64)
    da0, da2, dab = aR[0] - aR[1], aR[2] - aR[1], aR[3] - aR[1]
    db0, db2, dbb = bR[0] - bR[1], bR[2] - bR[1], bR[3] - bR[1]

    b0 = float(base[0])
    lp0c = _f32(np.float32(np.log(base[0])) + np.float32(np.log(pi[0])))

    # pad rows: benign values
    BP = NCORES * P
    ds_p = np.full((BP, NB), 0.5, np.float32)
    ds_p[:B] = ds
    t_p = np.ones((BP, 1), np.float32)
    t_p[:B] = t

    # --- 28-col compare-block patterns ---
    # groups: [den_f0, den_f1, num_f0, num_f1], each 7 cols:
    #   3 "+": P=ds[p], Q=ds[q];  3 "-": P=ds[q], Q=ds[p];  1 base: P=1,Q=0
    P28 = np.empty((BP, 28), np.float32)
    Q28 = np.empty((BP, 28), np.float32)
    K28 = np.zeros(28, np.float32)
    K28E = np.zeros(28, np.float32)
    for g in range(4):
        i = g % 2          # formula
        is_num = g >= 2
        fscale = float(fw[i]) if is_num else 1.0
        dp, dm, db_ = (db0, db2, dbb) if is_num else (da0, da2, dab)
        base1, basev = (bR[1], float(fw[i])) if is_num else (aR[1], 1.0)
        c = 7 * g
        for k in range(3):
            pj, qj = p_all[3 * i + k], q_all[3 * i + k]
            P28[:, c + k] = ds_p[:, pj]
            Q28[:, c + k] = ds_p[:, qj]
            P28[:, c + 3 + k] = ds_p[:, qj]
            Q28[:, c + 3 + k] = ds_p[:, pj]
            K28[c + k] = pv[3 * i + k] * dp * fscale
            K28[c + 3 + k] = pv[3 * i + k] * dm * fscale
            K28E[c + k] = pv[3 * i + k] * db_ * fscale
            K28E[c + 3 + k] = pv[3 * i + k] * db_ * fscale
        P28[:, c + 6] = 1.0
        Q28[:, c + 6] = 0.0
        if sel[i] == 1.0:
            K28[c + 6] = nv[i] * base1 * fscale
        else:
            K28[c + 6] = basev  # den base -> 1, num base -> fw  => col*fw = fw

    # device compare: (P + (-TOL)) > Q  /  == Q, all in f32
    pm = (P28 + np.float32(-TOL)).astype(np.float32)
    need_boundary = bool((pm[:, [c for c in range(28) if c % 7 != 6]]
                          == Q28[:, [c for c in range(28) if c % 7 != 6]]).any())

    ncol = NCOL_BND if need_boundary else NCOL_FAST
    Xf = np.zeros((BP, ncol), np.float32)
    ec = np.array([A[i, NB] + A[i, NB + 1] for i in range(nF)], np.float32) \
        - np.float32(KSEL)
    ds2 = np.full((BP, W2), -1e30, np.float32)
    ds2[:, :NB] = ds_p
    ds2[:, NB + 1:NB + 1 + NB] = ds_p
    a2row = np.zeros(W2, np.float32)
    a2row[:NB] = A[0, :NB]
    a2row[NB] = ec[0]
    a2row[NB + 1:NB + 1 + NB] = A[1, :NB]
    a2row[NB + 1 + NB] = ec[1]
    dsa = np.zeros((BP, W2), np.float32)
    dsa[:, :NB] = ds_p * A[0, :NB][None, :]
    dsa[:, NB + 1:NB + 1 + NB] = ds_p * A[1, :NB][None, :]

    Xf[:, C_DS2:C_DS2 + W2] = ds2
    Xf[:, C_A2:C_A2 + W2] = a2row[None, :]
    Xf[:, C_DSA:C_DSA + W2] = dsa
    Xf[:, C_P28:C_P28 + 28] = P28
    Xf[:, C_Q28:C_Q28 + 28] = Q28
    Xf[:, C_K28:C_K28 + 28] = K28[None, :]
    Xf[:, C_T:C_T + 1] = t_p
    Xf[:, C_NT:C_NT + 1] = -t_p
    Xf[:, C_TB:C_TB + 1] = -np.float32(b0) * t_p
    Xf[:, C_M1:C_M1 + 1] = -1.0
    Xf[:, C_PI:C_PI + 2] = pi[1:][None, :]
    Xf[:, C_MG:C_MG + 4] = np.full((1, 4), MAGIC, np.int32).view(np.float32)
    if need_boundary:
        Xf[:, C_KE:C_KE + 28] = K28E[None, :]

    cfg = (int(P), need_boundary, _f32(-1.0 / SIGMA), _f32(b0), lp0c)
    return cfg, Xf.reshape(NCORES, P, ncol)


def kernel(t, data_sample, pi, A, base, formula_weight, prob):
    global LAST_RESULT
    cfg, X = _prepare(t, data_sample, pi, A, base, formula_weight, prob)
    B = np.asarray(t).shape[0]

    cached = _BUILD_CACHE.get(cfg)
    if cached is None:
        cached = _build(cfg)
        _BUILD_CACHE[cfg] = cached
    nc, _ctx = cached

    in_maps = [{"x": np.ascontiguousarray(X[c])} for c in range(NCORES)]
    res = run_bass_kernel_spmd(nc, in_maps, core_ids=list(range(NCORES)))
    LAST_RESULT = res
    out = np.concatenate([res.results[c]["o"] for c in range(NCORES)], axis=0)
    return np.ascontiguousarray(out[:B]).astype(np.float32)
